# revision 13
# baseline (speedup 1.0000x reference)
"""Trainium2 Bass kernel for the DanQ-cat-attention model (v2).

Data-parallel over batch: 800 rows split across 8 NeuronCores (100 each),
weights replicated. BatchNorm batch stats are summed with an in-kernel
AllReduce. Everything else is core-local.

Per-core pipeline (feature-on-partition layouts, bf16 activations):
  A. Conv1d(4->320,k=26) as a K=104 matmul over im2col slabs (20 batches per
     DMA). The 64-row channel chunk packs two consecutive batches into one
     128-partition psum tile. MaxPool(13) reads conv PSUM via windowed
     reduce_max split between the DVE and Pool engines.
     gene = geneexpr @ gene_w.T is interleaved on the PE (K=19840 streamed).
  B. Per-timestep single-step LSTM (both dirs) = matmul [320->480] with
     i/g/o 128-row chunks + one packed 96-row i'/g'/o' chunk, fused
     sigmoid/tanh gating (forget gate drops out since c0=0). h kept bf16.
  C. BN stats via ones-matmuls + AllReduce (hidden under B); BN+ReLU fused
     into the PE transpose eviction.
  D. attn[t,b] = <h[t,b,:], gene[b,:]> via mult + ones-matmul; attn is
     broadcast to 128 partitions with a rank-1 PE matmul (not gpsimd);
     h *= attn in place.
  E. hid = relu(flat @ lin_w.T + b) as 45 per-timestep matmuls reading the
     weighted h directly (no flat materialization); lin_w.T is reordered
     host-side to [dchunk, t, 925] and streamed with a rolling prefetch
     that starts during stage B. out = hid @ out_w.T reduced on DVE.
"""

import ml_dtypes
import numpy as np

import concourse.bass as bass
import concourse.mybir as mybir
import concourse.tile as tile
from concourse import ap_utils, bacc
from concourse.bass import AP
from concourse.bass_utils import run_bass_kernel_spmd
from concourse.masks import make_identity

F32 = mybir.dt.float32
BF16 = mybir.dt.bfloat16
AX = mybir.AxisListType
AF = mybir.ActivationFunctionType
MUL = mybir.AluOpType.mult
ADD = mybir.AluOpType.add
SUB = mybir.AluOpType.subtract

N_CORES = 8
B_TOTAL = 800
L_IN = 600
CIN = 4
KW = 26
KK = CIN * KW  # 104
CO = 320
T = 45
POOL = 13
NPOS = 571  # conv positions consumed by the pool windows
HID = 160
BN_EPS = 1e-5

KG = 19840  # 19795 padded to 155*128
KGC = KG // 128
H2 = 925

SLAB = 20   # batches per im2col DMA slab
LWB = 8     # lin_w prefetch bufs (t-pairs in flight)
KB = 4      # gene k-chunks per DMA


def build_nc(n_cores: int, BL: int):
    BT = BL * T
    NTB = 10  # batches per N-tile in stages B/D (450 columns)
    nt_list = [(b0 * T, NTB * T, b0, NTB) for b0 in range(0, BL, NTB)]
    b_total = BL * n_cores
    n_slabs = BL // SLAB

    nc = bacc.Bacc("TRN2", target_bir_lowering=False, debug=False,
                   num_devices=n_cores)

    # ---- I/O ----
    x_l = nc.dram_tensor("x_l", [BL, CIN, L_IN], BF16, kind="ExternalInput")
    wcol = nc.dram_tensor("wcol", [KK, CO], BF16, kind="ExternalInput")
    convb = nc.dram_tensor("convb", [128, 3], F32, kind="ExternalInput")
    wg = nc.dram_tensor("wg", [2, 128, 3, 480], BF16, kind="ExternalInput")
    gbias = nc.dram_tensor("gbias", [2, 128, 6], F32, kind="ExternalInput")
    geT = nc.dram_tensor("geT", [128, KGC, BL], BF16, kind="ExternalInput")
    gwT = nc.dram_tensor("gwT", [128, KGC, CO], BF16, kind="ExternalInput")
    gbcols = nc.dram_tensor("gbcols", [128, 6], F32, kind="ExternalInput")
    lw01 = nc.dram_tensor("lw01", [128, 2, T, H2], BF16, kind="ExternalInput")
    lw2 = nc.dram_tensor("lw2", [64, T, H2], BF16, kind="ExternalInput")
    linb = nc.dram_tensor("linb", [1, H2], F32, kind="ExternalInput")
    outw = nc.dram_tensor("outw", [1, H2], F32, kind="ExternalInput")
    outb = nc.dram_tensor("outb", [1, 1], F32, kind="ExternalInput")
    y = nc.dram_tensor("y", [BL, 1], F32, kind="ExternalOutput")

    with tile.TileContext(nc) as tc:
        with (
            tc.tile_pool(name="const", bufs=1) as cst,
            tc.tile_pool(name="persist", bufs=1) as per,
            tc.tile_pool(name="dram", bufs=1, space="DRAM") as dram,
        ):
            # ---- constants ----
            wcol_sb = cst.tile([KK, CO], BF16)
            nc.sync.dma_start(wcol_sb[:], wcol.ap())
            convb_sb = cst.tile([128, 3], F32)
            nc.sync.dma_start(convb_sb[:], convb.ap())
            wg_sb = cst.tile([128, 2, 3, 480], BF16)
            nc.sync.dma_start(wg_sb[:, 0], wg.ap()[0])
            nc.sync.dma_start(wg_sb[:, 1], wg.ap()[1])
            gbias_sb = cst.tile([128, 2, 6], F32)
            nc.sync.dma_start(gbias_sb[:, 0], gbias.ap()[0])
            nc.sync.dma_start(gbias_sb[:, 1], gbias.ap()[1])
            gbcols_sb = cst.tile([128, 6], F32)
            nc.sync.dma_start(gbcols_sb[:], gbcols.ap())
            linb_bc = cst.tile([BL, H2], F32)
            nc.sync.dma_start(linb_bc[:], linb.ap().to_broadcast([BL, H2]))
            outw_bc = cst.tile([BL, H2], F32)
            nc.sync.dma_start(outw_bc[:], outw.ap().to_broadcast([BL, H2]))
            outb_col = cst.tile([BL, 1], F32)
            nc.sync.dma_start(outb_col[:], outb.ap().to_broadcast([BL, 1]))
            ones_k = cst.tile([128, 1], F32)
            nc.any.memset(ones_k[:], 1.0)
            ones_b = cst.tile([128, 1], BF16)
            nc.any.memset(ones_b[:], 1.0)
            ones_row = cst.tile([1, 128], BF16)
            nc.any.memset(ones_row[:], 1.0)
            eps_c = cst.tile([128, 1], F32)
            nc.any.memset(eps_c[:], BN_EPS)
            ident = cst.tile([128, 128], F32)
            make_identity(nc, ident[:])

            # ---- persistent activations ----
            hc0 = per.tile([128, BT], BF16)  # h dims 0..127
            hc1 = per.tile([128, BT], BF16)  # h dims 128..255
            hc2 = per.tile([64, BT], BF16)   # h dims 256..319
            geneT = per.tile([128, 3, BL], BF16)  # BN+ReLU'd gene, transposed
            gene_sb = per.tile([BL, CO], F32)
            gstats = per.tile([128, 6], F32)

            kb_list = list(range(0, KGC, KB))

            with (
                tc.tile_pool(name="psC1", bufs=1, space="PSUM") as psc1,
                tc.tile_pool(name="psS", bufs=1, space="PSUM") as pss,
                tc.tile_pool(name="wkc", bufs=4) as wkc,
                tc.tile_pool(name="stat", bufs=1) as stp,
            ):
              ps_gene = psc1.tile([BL, CO], F32, tag="pg")

              def emit_gene_batch(kb):
                  nkb = min(KB, KGC - kb)
                  ge_t = wkc.tile([128, KB, BL], BF16, tag="ge")
                  nc.sync.dma_start(ge_t[:, 0:nkb], geT.ap()[:, kb:kb + nkb])
                  gw_t = wkc.tile([128, KB, CO], BF16, tag="gw")
                  nc.sync.dma_start(gw_t[:, 0:nkb], gwT.ap()[:, kb:kb + nkb])
                  for j in range(nkb):
                      kc = kb + j
                      nc.tensor.matmul(ps_gene[:, :], ge_t[:, j], gw_t[:, j],
                                       start=(kc == 0), stop=(kc == KGC - 1))

              # =========== Stage A: conv + maxpool (gene interleaved) =======
              with tc.tile_pool(name="seqp", bufs=1) as seqp:
                seq = seqp.tile([128, 3, BT], BF16)
                seq_v = [seq[0:mn, mc].rearrange("p (b t) -> p b t", t=T)
                         for mc, mn in enumerate((128, 128, 64))]
                with (
                  tc.tile_pool(name="wka", bufs=2) as wka,
                  tc.tile_pool(name="pooltmp", bufs=6) as ptp,
                  tc.tile_pool(name="psA", bufs=2, space="PSUM") as psa,
                  tc.tile_pool(name="psA2", bufs=1, space="PSUM") as psa2,
                ):
                  # t=0 pool window is all left-padding -> exactly 0
                  for mc in range(3):
                      nc.vector.memset(seq_v[mc][:, :, 0:1], 0.0)

                  red_idx = 0

                  def pool_tile(ps):
                      """Windowed max of psum [128, NPOS] -> tmp [128, 44].
                      Two paths to balance DVE vs ACT load (gpsimd can't read
                      PSUM, and InstPool is DVE-only on TRN2):
                        1) DVE reduce straight from PSUM       (DVE ~770ns)
                        2) ACT copy->SBUF bf16 (~650ns), then DVE reduce in
                           4x mode (~330ns)
                      """
                      nonlocal red_idx
                      r = red_idx % 50
                      red_idx += 1
                      if r < 27:
                          tmp = ptp.tile([128, T - 1], F32, tag="pt")
                          big = ps[:, 12:NPOS].rearrange("p (t k) -> p t k",
                                                         k=POOL)
                          nc.vector.reduce_max(tmp[:, 0:1], ps[:, 0:12],
                                               axis=AX.X)
                          nc.vector.reduce_max(tmp[:, 1:T - 1], big, axis=AX.X)
                          return tmp
                      sb = ptp.tile([128, NPOS], BF16, tag="sb")
                      nc.scalar.activation(sb[:], ps[:], AF.Copy)
                      tmp = ptp.tile([128, T - 1], BF16, tag="ptb")
                      big = sb[:, 12:NPOS].rearrange("p (t k) -> p t k", k=POOL)
                      nc.vector.reduce_max(tmp[:, 0:1], sb[:, 0:12], axis=AX.X)
                      nc.vector.reduce_max(tmp[:, 1:T - 1], big, axis=AX.X)
                      return tmp

                  ki = 0
                  ps2 = None
                  for s in range(n_slabs):
                      b0 = s * SLAB
                      xrep = wka.tile([KK, SLAB, NPOS], BF16, tag="xrep")
                      for c in range(CIN):
                          src = AP(x_l.ap().tensor, (b0 * CIN + c) * L_IN,
                                   [[1, KW], [CIN * L_IN, SLAB], [1, NPOS]])
                          nc.sync.dma_start(xrep[c * KW:(c + 1) * KW], src)
                      xflat = xrep.rearrange("p b n -> p (b n)")
                      for bl in range(SLAB):
                          b = b0 + bl
                          if b % 2 == 0 and ki < len(kb_list):
                              emit_gene_batch(kb_list[ki])
                              ki += 1
                          cols = slice(bl * NPOS, (bl + 1) * NPOS)
                          xcols = xflat[:, cols]
                          for mc in range(2):
                              m0 = mc * 128
                              ps = psa.tile([128, NPOS], F32, tag="cps")
                              for p0, pn in ((0, 512), (512, NPOS - 512)):
                                  nc.tensor.matmul(
                                      ps[:, p0:p0 + pn],
                                      wcol_sb[:, m0:m0 + 128],
                                      xcols[:, p0:p0 + pn],
                                      start=True, stop=True)
                              tmp = pool_tile(ps)
                              nc.scalar.activation(
                                  seq_v[mc][:, b, 1:T], tmp[:],
                                  AF.Relu, bias=convb_sb[:, mc:mc + 1])
                          # channels 256:320: pack 2 batches per psum tile
                          half = bl % 2
                          if half == 0:
                              ps2 = psa2.tile([128, NPOS], F32, tag="c2")
                          for p0, pn in ((0, 512), (512, NPOS - 512)):
                              nc.tensor.matmul(
                                  ps2[64 * half:64 * half + 64, p0:p0 + pn],
                                  wcol_sb[:, 256:320],
                                  xcols[:, p0:p0 + pn],
                                  start=True, stop=True)
                          if half == 1:
                              tmp = pool_tile(ps2)
                              nc.scalar.activation(
                                  seq_v[2][:, b - 1, 1:T], tmp[0:64],
                                  AF.Relu, bias=convb_sb[0:64, 2:3])
                              nc.scalar.activation(
                                  seq_v[2][:, b, 1:T], tmp[64:128],
                                  AF.Relu, bias=convb_sb[0:64, 2:3])
                  while ki < len(kb_list):
                      emit_gene_batch(kb_list[ki])
                      ki += 1

                  # ---- C2a: BN stats + AllReduce trigger (hidden under B) --
                  gsq = stp.tile([BL, CO], F32)
                  nc.scalar.activation(gene_sb[:], ps_gene[:], AF.Copy)
                  nc.scalar.square(gsq[:], ps_gene[:])
                  stats = stp.tile([128, 6], F32)
                  nc.vector.memset(stats[:], 0.0)
                  for c in range(3):
                      cn = (128, 128, 64)[c]
                      ps_s = pss.tile([128, 1], F32, tag="cstat")
                      nc.tensor.matmul(ps_s[0:cn, :],
                                       gene_sb[:, c * 128:c * 128 + cn],
                                       ones_k[0:BL, :], start=True, stop=True)
                      nc.scalar.activation(stats[0:cn, c:c + 1], ps_s[0:cn, :],
                                           AF.Copy)
                      ps_q = pss.tile([128, 1], F32, tag="cstat")
                      nc.tensor.matmul(ps_q[0:cn, :],
                                       gsq[:, c * 128:c * 128 + cn],
                                       ones_k[0:BL, :], start=True, stop=True)
                      nc.scalar.activation(stats[0:cn, 3 + c:4 + c], ps_q[0:cn, :],
                                           AF.Copy)
                  cc_in = dram.tile([128, 6], F32)
                  cc_out = dram.tile([128, 6], F32)
                  nc.scalar.dma_start(cc_in[:], stats[:])
                  nc.gpsimd.collective_compute(
                      "AllReduce", ADD,
                      replica_groups=[list(range(n_cores))],
                      ins=[cc_in.opt()], outs=[cc_out.opt()],
                  )
                  nc.gpsimd.dma_start(gstats[:], cc_out[:])

                # ---- lin_w prefetch pool spans stages B..E ----
                with tc.tile_pool(name="lwp", bufs=LWB) as lwp:
                    n_tp = (T + 1) // 2  # 23 t-pairs (last is a single t)
                    lw_tiles = {}

                    def emit_lw(tp):
                        t0 = tp * 2
                        ntp = min(2, T - t0)
                        a = lwp.tile([128, 2, 2, H2], BF16, tag="lw01")
                        nc.sync.dma_start(a[:, :, 0:ntp],
                                          lw01.ap()[:, :, t0:t0 + ntp])
                        b = lwp.tile([64, 2, H2], BF16, tag="lw2")
                        nc.sync.dma_start(b[:, 0:ntp], lw2.ap()[:, t0:t0 + ntp])
                        lw_tiles[tp] = (a, b)

                    lw_emitted = 0

                    # ---- Stage B: gates + gating ----
                    with (
                        tc.tile_pool(name="wkb", bufs=3) as wkb,
                        tc.tile_pool(name="psB", bufs=6, space="PSUM") as psb,
                    ):
                        m_chunks = ((0, 128), (128, 128), (256, 128), (384, 96))
                        for d in range(2):
                            for n0, nn, _, _ in nt_list:
                                if lw_emitted < LWB:
                                    emit_lw(lw_emitted)
                                    lw_emitted += 1
                                ps_i = psb.tile([128, 450], F32, tag="g")
                                ps_g = psb.tile([128, 450], F32, tag="g")
                                ps_o = psb.tile([128, 450], F32, tag="g")
                                ps_x = psb.tile([128, 450], F32, tag="g")
                                for (m0, mn), pst in zip(
                                        m_chunks, (ps_i, ps_g, ps_o, ps_x)):
                                    for kc in range(3):
                                        kn = (128, 128, 64)[kc]
                                        nc.tensor.matmul(
                                            pst[0:mn, 0:nn],
                                            wg_sb[0:kn, d, kc, m0:m0 + mn],
                                            seq[0:kn, kc, n0:n0 + nn],
                                            start=(kc == 0), stop=(kc == 2),
                                        )
                                bia = gbias_sb[:, d]
                                si = wkb.tile([128, 450], BF16, tag="si")
                                tg = wkb.tile([128, 450], BF16, tag="tg")
                                so = wkb.tile([128, 450], BF16, tag="so")
                                tc_ = wkb.tile([128, 450], BF16, tag="tc")
                                s3i = wkb.tile([32, 450], BF16, tag="s3i")
                                s3g = wkb.tile([32, 450], BF16, tag="s3g")
                                s3o = wkb.tile([32, 450], BF16, tag="s3o")
                                t3 = wkb.tile([32, 450], BF16, tag="t3")
                                nc.scalar.activation(si[:, 0:nn], ps_i[:, 0:nn],
                                                     AF.Sigmoid, bias=bia[:, 0:1])
                                nc.scalar.activation(tg[:, 0:nn], ps_g[:, 0:nn],
                                                     AF.Tanh, bias=bia[:, 1:2])
                                nc.scalar.activation(so[:, 0:nn], ps_o[:, 0:nn],
                                                     AF.Sigmoid, bias=bia[:, 2:3])
                                nc.scalar.activation(s3i[:, 0:nn],
                                                     ps_x[0:32, 0:nn],
                                                     AF.Sigmoid,
                                                     bias=bia[0:32, 3:4])
                                nc.scalar.activation(s3g[:, 0:nn],
                                                     ps_x[32:64, 0:nn],
                                                     AF.Tanh,
                                                     bias=bia[0:32, 4:5])
                                nc.scalar.activation(s3o[:, 0:nn],
                                                     ps_x[64:96, 0:nn],
                                                     AF.Sigmoid,
                                                     bias=bia[0:32, 5:6])
                                nc.vector.tensor_tensor(tc_[:, 0:nn], si[:, 0:nn],
                                                        tg[:, 0:nn], MUL)
                                nc.vector.tensor_tensor(t3[:, 0:nn],
                                                        s3i[:, 0:nn],
                                                        s3g[:, 0:nn], MUL)
                                nc.scalar.activation(tc_[:, 0:nn], tc_[:, 0:nn],
                                                     AF.Tanh)
                                nc.scalar.activation(t3[:, 0:nn], t3[:, 0:nn],
                                                     AF.Tanh)
                                if d == 0:
                                    nc.vector.tensor_tensor(
                                        hc0[:, n0:n0 + nn], tc_[:, 0:nn],
                                        so[:, 0:nn], MUL)
                                    nc.vector.tensor_tensor(
                                        hc1[0:32, n0:n0 + nn], t3[:, 0:nn],
                                        s3o[:, 0:nn], MUL)
                                else:
                                    for q in range(3):
                                        nc.vector.tensor_tensor(
                                            hc1[32 + 32 * q:64 + 32 * q,
                                                n0:n0 + nn],
                                            tc_[32 * q:32 * q + 32, 0:nn],
                                            so[32 * q:32 * q + 32, 0:nn], MUL)
                                    nc.vector.tensor_tensor(
                                        hc2[0:32, n0:n0 + nn], tc_[96:128, 0:nn],
                                        so[96:128, 0:nn], MUL)
                                    nc.vector.tensor_tensor(
                                        hc2[32:64, n0:n0 + nn], t3[:, 0:nn],
                                        s3o[:, 0:nn], MUL)

                        # emit the rest of the lin_w stream; the sync queue
                        # self-paces on pool-slot release during stage E
                        while lw_emitted < n_tp:
                            emit_lw(lw_emitted)
                            lw_emitted += 1

                        # ---- C2b: BN scale/bias + fused transpose ----
                        mean = stp.tile([128, 3], F32)
                        var = stp.tile([128, 3], F32)
                        scl = stp.tile([128, 3], F32)
                        nbi = stp.tile([128, 3], F32)
                        inv_b = 1.0 / float(b_total)
                        nc.scalar.activation(mean[:], gstats[:, 0:3], AF.Copy,
                                             scale=inv_b)
                        nc.scalar.activation(var[:], gstats[:, 3:6], AF.Copy,
                                             scale=inv_b)
                        msq = stp.tile([128, 3], F32)
                        nc.scalar.square(msq[:], mean[:])
                        nc.vector.tensor_tensor(var[:], var[:], msq[:], SUB)
                        std = stp.tile([128, 3], F32)
                        nc.scalar.activation(std[:], var[:], AF.Sqrt,
                                             bias=eps_c[:])
                        rstd = stp.tile([128, 3], F32)
                        nc.vector.reciprocal(rstd[:], std[:])
                        nc.vector.tensor_tensor(scl[:], gbcols_sb[:, 0:3],
                                                rstd[:], MUL)
                        nc.vector.tensor_tensor(nbi[:], mean[:], scl[:], MUL)
                        nc.vector.tensor_tensor(nbi[:], gbcols_sb[:, 3:6],
                                                nbi[:], SUB)
                        for c in range(3):
                            cn = (128, 128, 64)[c]
                            ps_t = pss.tile([128, BL], F32, tag="cstat")
                            nc.tensor.transpose(ps_t[0:cn, :],
                                                gene_sb[:, c * 128:c * 128 + cn],
                                                ident[0:BL, 0:BL])
                            nc.scalar.activation(geneT[0:cn, c, :],
                                                 ps_t[0:cn, :], AF.Relu,
                                                 bias=nbi[0:cn, c:c + 1],
                                                 scale=scl[0:cn, c:c + 1])

                    # ====== Stage D: attention; h *= attn in place ======
                    with (
                        tc.tile_pool(name="wkd", bufs=2) as wkd,
                        tc.tile_pool(name="psD", bufs=2, space="PSUM") as psd,
                    ):
                        for n0, nn, b0, nb in nt_list:
                            prod = wkd.tile([128, 3, 450], BF16, tag="prod")
                            for c, (htile, cn) in enumerate(
                                    ((hc0, 128), (hc1, 128), (hc2, 64))):
                                nc.vector.tensor_tensor(
                                    prod[0:cn, c, 0:nn].rearrange(
                                        "p (b t) -> p b t", t=T),
                                    htile[0:cn, n0:n0 + nn].rearrange(
                                        "p (b t) -> p b t", t=T),
                                    geneT[0:cn, c, b0:b0 + nb][:, :, None]
                                    .to_broadcast([cn, nb, T]),
                                    MUL,
                                )
                            ps_a = psd.tile([1, 450], F32, tag="ps_a")
                            for c in range(3):
                                cn = (128, 128, 64)[c]
                                nc.tensor.matmul(ps_a[0:1, 0:nn],
                                                 ones_b[0:cn, :],
                                                 prod[0:cn, c, 0:nn],
                                                 start=(c == 0), stop=(c == 2))
                            attn_row = wkd.tile([1, 450], BF16, tag="arow")
                            nc.scalar.activation(attn_row[:, 0:nn],
                                                 ps_a[:, 0:nn], AF.Copy)
                            ps_bc = psd.tile([128, 450], F32, tag="ps_bc")
                            nc.tensor.matmul(ps_bc[:, 0:nn], ones_row[:, :],
                                             attn_row[:, 0:nn],
                                             start=True, stop=True)
                            attn_bc = wkd.tile([128, 450], BF16, tag="abc")
                            nc.scalar.activation(attn_bc[:, 0:nn],
                                                 ps_bc[:, 0:nn], AF.Copy)
                            for htile, cn in ((hc0, 128), (hc1, 128),
                                              (hc2, 64)):
                                nc.vector.tensor_tensor(
                                    htile[0:cn, n0:n0 + nn],
                                    htile[0:cn, n0:n0 + nn],
                                    attn_bc[0:cn, 0:nn], MUL)

                    # ====== Stage E: hid = relu(flat @ lin_w.T + b) ======
                    with (
                        tc.tile_pool(name="wke", bufs=1) as wke,
                        tc.tile_pool(name="psE", bufs=1, space="PSUM") as pse,
                    ):
                        ps_hid = pse.tile([BL, H2], F32, tag="ph")
                        for t in range(T):
                            tp, ti = divmod(t, 2)
                            a, btile = lw_tiles[tp]
                            for c in range(3):
                                cn = (128, 128, 64)[c]
                                htile = (hc0, hc1, hc2)[c]
                                lhsT = htile[0:cn, t::T]
                                rhs = (a[:, c, ti] if c < 2
                                       else btile[0:cn, ti])
                                for j0, jn in ((0, 512), (512, H2 - 512)):
                                    nc.tensor.matmul(
                                        ps_hid[:, j0:j0 + jn],
                                        lhsT, rhs[:, j0:j0 + jn],
                                        start=(t == 0 and c == 0),
                                        stop=(t == T - 1 and c == 2))
                        hid = wke.tile([BL, H2], F32, tag="hid")
                        nc.vector.tensor_tensor(hid[:], ps_hid[:],
                                                linb_bc[:], ADD)
                        nc.scalar.activation(hid[:], hid[:], AF.Relu)
                        hw = wke.tile([BL, H2], F32, tag="hw")
                        nc.vector.tensor_tensor(hw[:], hid[:], outw_bc[:], MUL)
                        y_sb = wke.tile([BL, 1], F32, tag="ysb")
                        nc.vector.reduce_sum(y_sb[:], hw[:], axis=AX.X)
                        nc.vector.tensor_tensor(y_sb[:], y_sb[:],
                                                outb_col[:], ADD)
                        nc.sync.dma_start(y.ap(), y_sb[:])

    nc.compile()
    return nc


def make_in_maps(inputs, n_cores: int, BL: int):
    """Host-side prep: shard + transpose + pad + reorder weights."""
    f32 = np.float32
    bf16 = ml_dtypes.bfloat16

    def pad_rows(a, n):
        return np.pad(a, ((0, n - a.shape[0]),) + ((0, 0),) * (a.ndim - 1))

    conv_w = np.asarray(inputs["conv_w"], f32)
    wcol = np.ascontiguousarray(
        conv_w.transpose(1, 2, 0).reshape(KK, CO)).astype(bf16)
    convb = np.ascontiguousarray(
        pad_rows(np.asarray(inputs["conv_b"], f32), 384).reshape(3, 128).T)

    def gate_prep(w_ih, b_ih, b_hh):
        W = np.asarray(w_ih, f32).T  # [320, 640]
        b = (np.asarray(b_ih, f32) + np.asarray(b_hh, f32))  # [640]
        cols = (list(range(0, 128)) + list(range(320, 448)) +
                list(range(480, 608)) + list(range(128, 160)) +
                list(range(448, 480)) + list(range(608, 640)))
        Wr = W[:, cols]  # [320, 480]
        br = b[cols]  # [480]
        Wr = pad_rows(Wr, 384).reshape(3, 128, 480).transpose(1, 0, 2).astype(bf16)
        # bias cols: i(0:128), g(0:128), o(0:128), i'/g'/o' each at rows 0:32
        bc = np.zeros((128, 6), np.float32)
        bc[:, 0] = br[0:128]
        bc[:, 1] = br[128:256]
        bc[:, 2] = br[256:384]
        bc[0:32, 3] = br[384:416]
        bc[0:32, 4] = br[416:448]
        bc[0:32, 5] = br[448:480]
        return np.ascontiguousarray(Wr), np.ascontiguousarray(bc)

    wgf, bgf = gate_prep(inputs["w_ih_f"], inputs["b_ih_f"], inputs["b_hh_f"])
    wgb, bgb = gate_prep(inputs["w_ih_b"], inputs["b_ih_b"], inputs["b_hh_b"])
    wg = np.stack([wgf, wgb])  # [2, 128, 3, 480]
    gbias = np.stack([bgf, bgb])  # [2, 128, 6]

    gene_w = np.asarray(inputs["gene_w"], f32)  # [320, 19795]
    gwT = np.ascontiguousarray(pad_rows(np.ascontiguousarray(gene_w.T), KG)
                               .reshape(KGC, 128, CO).transpose(1, 0, 2)).astype(bf16)
    gamma = pad_rows(np.asarray(inputs["bn_gamma"], f32), 384).reshape(3, 128).T
    beta = pad_rows(np.asarray(inputs["bn_beta"], f32), 384).reshape(3, 128).T
    gbcols = np.ascontiguousarray(np.concatenate([gamma, beta], 1))  # [128, 6]

    lin_w = np.asarray(inputs["lin_w"], f32)  # [925, 14400]
    lwT = np.ascontiguousarray(lin_w.T).reshape(T, CO, H2)  # [45, 320, 925]
    lw01 = np.ascontiguousarray(
        np.stack([lwT[:, 0:128, :], lwT[:, 128:256, :]], axis=0)
        .transpose(2, 0, 1, 3)).astype(bf16)  # [128, 2, 45, 925]
    lw2 = np.ascontiguousarray(
        lwT[:, 256:320, :].transpose(1, 0, 2)).astype(bf16)  # [64, 45, 925]
    linb = np.asarray(inputs["lin_b"], f32).reshape(1, H2)
    outw = np.asarray(inputs["out_w"], f32).reshape(1, H2)
    outb = np.asarray(inputs["out_b"], f32).reshape(1, 1)

    x = np.asarray(inputs["x"], f32)
    ge = np.asarray(inputs["geneexpr"], f32)

    shared = dict(wcol=wcol, convb=convb, wg=wg, gbias=gbias, gwT=gwT,
                  gbcols=gbcols, lw01=lw01, lw2=lw2, linb=linb, outw=outw,
                  outb=outb)
    in_maps = []
    for i in range(n_cores):
        sl = slice(i * BL, (i + 1) * BL)
        geT = np.ascontiguousarray(pad_rows(np.ascontiguousarray(ge[sl].T), KG)
                                   .reshape(KGC, 128, BL).transpose(1, 0, 2)).astype(bf16)
        m = dict(shared)
        m["x_l"] = np.ascontiguousarray(x[sl]).astype(bf16)
        m["geT"] = geT
        in_maps.append(m)
    return in_maps


_NC_CACHE = {}


def _get_nc(n_cores, BL):
    key = (n_cores, BL)
    if key not in _NC_CACHE:
        _NC_CACHE[key] = build_nc(n_cores, BL)
    return _NC_CACHE[key]


def kernel(**inputs) -> np.ndarray:
    BL = B_TOTAL // N_CORES
    nc = _get_nc(N_CORES, BL)
    in_maps = make_in_maps(inputs, N_CORES, BL)
    res = run_bass_kernel_spmd(nc, in_maps, list(range(N_CORES)))
    return np.concatenate([res.results[i]["y"] for i in range(N_CORES)], axis=0)
